# revision 46
# baseline (speedup 1.0000x reference)
"""Trainium2 Bass kernel for nn_GCBlock (GNN message passing).

Strategy (8 NeuronCores, SPMD, no collectives):
- Atoms padded to a multiple of 128; 128-atom "windows" greedy-assigned to
  cores balancing pair count. Pairs are grouped by source-atom window.
- Per 128-pair tile, a one-hot matrix (pairs x window-atoms) is built on
  device with is_equal; TensorEngine matmuls use it both to gather i-side
  per-atom projections (P1proj/P3proj tables) and to segment-sum pair
  results back to atoms (transposed accumulator in PSUM).
- j-side neighbor features are host-pre-gathered (halo exchange per the
  sharding hint) and shipped feature-major in fp16 inside a packed per-tile
  blob (single DMA per tile -> single sem wait per consumer).
- Phase B (per-atom epilogue) runs on device per window in fp32.
- fp16 matmul operands with fp32 PSUM accumulation keep rel-err ~4e-3.
"""
import sys
sys.path.insert(0, "/opt/trn_rl_repo")

import numpy as np
from contextlib import ExitStack

import concourse.bass as bass
import concourse.tile as tile
import concourse.mybir as mybir
from concourse import bacc
from concourse.bass_utils import run_bass_kernel_spmd

f16 = mybir.dt.float16
f32 = mybir.dt.float32
ALU = mybir.AluOpType
AF = mybir.ActivationFunctionType
AX = mybir.AxisListType

# Problem sizes (overridable for mini-tests)
N_ATOMS = 25000
N_PAIRS = 500000
C = 64
PI_N = 32
NB = 8
WIN = 128
NCORES = 8

BT = 8  # tiles per output DMA block

LAST_EXEC_NS = None  # set by kernel() when GCB_TRACE=1
LAST_RES = None


# ---------------------------------------------------------------- host prep

def plan_shards(ind_2, n_atoms, ncores):
    na_pad = ((n_atoms + WIN - 1) // WIN) * WIN
    nwin_g = na_pad // WIN
    ii = ind_2[:, 0].astype(np.int64)
    wid = ii // WIN
    counts = np.bincount(wid, minlength=nwin_g)
    order = np.argsort(wid, kind="stable")
    win_start = np.zeros(nwin_g + 1, np.int64)
    np.cumsum(counts, out=win_start[1:])

    w_order = np.argsort(-counts, kind="stable")
    loads = np.zeros(ncores, np.int64)
    core_wins = [[] for _ in range(ncores)]
    for w in w_order:
        k = int(np.argmin(loads))
        core_wins[k].append(int(w))
        loads[k] += counts[w]
    nwin = max(len(x) for x in core_wins)
    for k in range(ncores):
        core_wins[k].sort(key=lambda w: -counts[w])
        core_wins[k] += [-1] * (nwin - len(core_wins[k]))

    T_m = np.zeros(nwin, np.int64)
    for m in range(nwin):
        for k in range(ncores):
            w = core_wins[k][m]
            n = counts[w] if w >= 0 else 0
            T_m[m] = max(T_m[m], (n + 127) // 128)
        T_m[m] = max(T_m[m], 1)
    TT = int(T_m.sum())

    pair_sel = np.full((ncores, TT * 128), -1, np.int64)
    for k in range(ncores):
        off = 0
        for m in range(nwin):
            w = core_wins[k][m]
            npair = int(counts[w]) if w >= 0 else 0
            if npair:
                pair_sel[k, off:off + npair] = order[win_start[w]:win_start[w] + npair]
            off += int(T_m[m]) * 128
    tile_slot = np.repeat(np.arange(nwin), T_m)
    return dict(NWIN=nwin, T_m=T_m, TT=TT, core_wins=core_wins,
                pair_sel=pair_sel, tile_slot=tile_slot, NA_PAD=na_pad)


def prep_core_inputs(plan, k, ind_2, p1_pad, p3_pad, d3, basis, W_pi, b_pi, wi_pix):
    """Packed per-tile blob + per-window tables for core k (all fp16)."""
    NWIN, T_m, TT = plan["NWIN"], plan["T_m"], plan["TT"]
    sel = plan["pair_sel"][k]
    valid = sel >= 0
    s = np.where(valid, sel, 0)
    ii = ind_2[s, 0].astype(np.int64)
    jj = ind_2[s, 1].astype(np.int64)
    win_of_pair = np.repeat(
        np.array([plan["core_wins"][k][m] for m in plan["tile_slot"]]), 128)
    ii_rel = np.where(valid, ii - win_of_pair * WIN, -1).astype(np.float16)

    p1j = np.where(valid[:, None], p1_pad[jj], 0).astype(np.float16)
    p3j = np.where(valid[:, None, None], p3_pad[jj], 0).astype(np.float16)
    d3t = np.where(valid[:, None], d3[s], 0).astype(np.float16)
    bas = np.where(valid[:, None], basis[s], 0).astype(np.float16)

    blob = np.zeros((TT, 128, 400), np.float16)
    blob[:, 0:64, 0:128] = p1j.reshape(TT, 128, C).transpose(0, 2, 1)
    blob[:, 64:128, 0:384] = (
        p3j.reshape(TT, 128, 3, C).transpose(0, 3, 2, 1).reshape(TT, C, 384))
    blob[:, 0:3, 128:256] = d3t.reshape(TT, 128, 3).transpose(0, 2, 1)
    blob[:, 4, 128:256] = ii_rel.reshape(TT, 128)
    blob[:, :, 384:392] = bas.reshape(TT, 128, NB)
    blob[:, :, 392] = ii_rel.reshape(TT, 128)

    wtab = np.zeros((NWIN, 128, 448), np.float16)
    for m in range(NWIN):
        w = plan["core_wins"][k][m]
        if w < 0:
            continue
        a0 = w * WIN
        wtab[m, :, 0:256] = (p1_pad[a0:a0 + WIN] @ W_pi[:C] + b_pi).astype(np.float16)
        wtab[m, :, 256:448] = np.einsum(
            "axc,cd->axd", p3_pad[a0:a0 + WIN], wi_pix).reshape(WIN, 3 * C)
    return blob, wtab


def prep_consts(W_pi, b_pi, W_ii, W_pp, wi_pix, wj_pix, W_ppx,
                wi_dot, wj_dot, W_pp1, b_pp1):
    c16 = np.zeros((128, 897), np.float16)
    c16[0:64, 0:256] = W_pi[C:]
    c16[0:PI_N, 256:384] = W_ii
    c16[0:64, 384:448] = wj_pix
    c16[64:128, 384:448] = wj_pix  # copy at base partition 64 (for p3jT lhsT)
    c16[0:3, 448:640] = np.kron(np.eye(3, dtype=np.float32), np.ones((1, C)))
    c16[:, 640:768] = np.arange(128, dtype=np.float16)[None, :]
    c16[:, 769:897] = np.eye(128, dtype=np.float16)
    c16 = np.concatenate([c16, np.zeros((128, 512), np.float16)], axis=1)

    c32 = np.zeros((128, 641), np.float32)
    c32[:, 0:64] = W_pp
    c32[0:64, 64:128] = W_ppx
    c32[64:128, 64:128] = W_ppx   # base-64 copies for accS[64:128] operands
    c32[0:64, 128:192] = W_ppx @ wi_dot
    c32[64:128, 128:192] = W_ppx @ wi_dot
    c32[0:64, 192:256] = W_ppx @ wj_dot
    c32[64:128, 192:256] = W_ppx @ wj_dot
    c32[:, 256:384] = W_pp1
    c32[:, 384] = b_pp1
    c32[:, 385:513] = np.eye(128, dtype=np.float32)
    c32[0:64, 513:641] = W_pp1[64:128, :]  # bottom half at base partition 0
    return c16, c32


# ------------------------------------------------------------- device build

def build_kernel(NWIN, TT, tile_slot, T_m):
    import os
    ablate = set(os.environ.get("GCB_ABLATE", "").split(","))
    nc = bacc.Bacc("TRN2", target_bir_lowering=False, debug=False)
    blob_d = nc.dram_tensor("blob", [TT, 128, 400], f16, kind="ExternalInput")
    wtab_d = nc.dram_tensor("wtab", [NWIN, 128, 448], f16, kind="ExternalInput")
    c16_d = nc.dram_tensor("c16", [128, 1409], f16, kind="ExternalInput")
    c32_d = nc.dram_tensor("c32", [128, 641], f32, kind="ExternalInput")
    i1_d = nc.dram_tensor("i1_out", [TT * 128, 128], f32, kind="ExternalOutput")
    ix_d = nc.dram_tensor("ix_out", [TT * 128, 192], f32, kind="ExternalOutput")
    pb_d = nc.dram_tensor("pb_out", [NWIN, 128, 320], f32, kind="ExternalOutput")

    # window start tile index
    w_start = np.zeros(NWIN + 1, np.int64)
    np.cumsum(T_m, out=w_start[1:])

    with tile.TileContext(nc) as tc, ExitStack() as ctx:
        cpool = ctx.enter_context(tc.tile_pool(name="consts", bufs=1))
        sb = ctx.enter_context(tc.tile_pool(name="sb", bufs=3))
        sb2 = ctx.enter_context(tc.tile_pool(name="sb2", bufs=2))
        ps = ctx.enter_context(tc.tile_pool(name="ps", bufs=5, space="PSUM"))
        psw = ctx.enter_context(tc.tile_pool(name="psw", bufs=2, space="PSUM"))

        c16 = cpool.tile([128, 1409], f16, name="c16t")
        nc.sync.dma_start(c16[:], c16_d[:])
        c32 = cpool.tile([128, 641], f32, name="c32t")
        nc.sync.dma_start(c32[:], c32_d[:])
        W_pi_bot = c16[0:64, 0:256]
        W_ii = c16[0:PI_N, 256:384]
        wj_pix_hi = c16[64:128, 384:448]
        kron3 = c16[0:3, 448:640]
        iota_row = c16[:, 640:768]
        id16 = c16[:, 769:897]
        W_pp = c32[:, 0:64]
        W_ppx = c32[0:64, 64:128]
        Wi2 = c32[0:64, 128:192]
        Wj2 = c32[0:64, 192:256]
        W_pp1_top = c32[0:64, 256:384]
        W_pp1_bot = c32[0:64, 513:641]
        b_pp1 = c32[:, 384:385]
        id32 = c32[:, 385:513]

        i1blk = None
        ixblk = None
        blk0 = 0

        for m in range(NWIN):
            wt = sb2.tile([128, 448], f16, name="wt")
            nc.sync.dma_start(wt[:], wtab_d[m])
            acc = psw.tile([128, 512], f32, name="acc")
            # zero the whole acc region (start=True clears has_written; the
            # c16 tail cols are all zeros so the matmul writes zeros)
            nc.tensor.matmul(acc[:], c16[0:1, 897:1025], c16[0:1, 897:1409],
                             start=True, stop=False)
            for t in range(int(w_start[m]), int(w_start[m + 1])):
                first = t == int(w_start[m])
                last = t == int(w_start[m + 1]) - 1
                bt = sb.tile([128, 400], f16, name="bt")
                nc.sync.dma_start(bt[:], blob_d[t])
                p1jT = bt[0:64, 0:128]
                d3T = bt[0:3, 128:256]
                ii_row = bt[4:5, 128:256]
                basis = bt[:, 384:392]
                ii_col = bt[:, 392:393]

                O = sb.tile([128, 128], f16, name="O")
                nc.vector.tensor_tensor(
                    O[:], iota_row, ii_col.to_broadcast((128, 128)), ALU.is_equal)
                otp = ps.tile([128, 128], f16, name="otp", tag="pa")
                nc.tensor.transpose(otp[:], O[:], id16)
                OT = sb.tile([128, 128], f16, name="OT")
                nc.scalar.activation(OT[:], otp[:], AF.Copy)

                inter_ps = ps.tile([128, 256], f32, name="inter_ps", tag="pa")
                nc.tensor.matmul(inter_ps[:], OT[:], wt[:, 0:256],
                                 start=True, stop=False)
                nc.tensor.matmul(inter_ps[:], p1jT, W_pi_bot,
                                 start=False, stop=True)
                inter_t = sb.tile([128, 256], f16, name="inter_t")
                nc.scalar.activation(inter_t[:], inter_ps[:], AF.Tanh)

                prod = sb.tile([128, 256], f16, name="prod")
                nc.vector.tensor_tensor(
                    prod[:], inter_t[:],
                    basis[:, None, :].to_broadcast((128, PI_N, NB)), ALU.mult)
                i1_pre = sb.tile([128, PI_N], f32, name="i1_pre")
                nc.vector.tensor_reduce(
                    i1_pre[:], prod[:].rearrange("p (a b) -> p a b", b=NB),
                    AX.X, ALU.add)
                ipp = ps.tile([PI_N, 128], f32, name="ipp", tag="pa")
                nc.tensor.transpose(ipp[:], i1_pre[:], id32)
                i1preT = sb.tile([PI_N, 128], f16, name="i1preT")
                nc.scalar.activation(i1preT[:], ipp[:], AF.Copy)
                i1_ps = ps.tile([128, 128], f32, name="i1_ps", tag="pa")
                nc.tensor.matmul(i1_ps[:], i1preT[:], W_ii, start=True, stop=True)

                if i1blk is None:
                    i1blk = sb2.tile([128, BT, 128], f16, name="i1blk")
                    ixblk = sb2.tile([128, BT, 192], f16, name="ixblk")
                    blk0 = t
                b = t - blk0
                nc.scalar.activation(i1blk[:, b, :], i1_ps[:], AF.Tanh)

                # full-region start clears has_written for all 192 cols;
                # x-blocks accumulate; full-region d3 matmul closes the group
                p3m_ps = ps.tile([128, 192], f32, name="p3m_ps", tag="pa")
                nc.tensor.matmul(p3m_ps[:], OT[:], wt[:, 256:448],
                                 start=True, stop=False)
                for x in range(3):
                    p3jT_x = bt[64:128, 128 * x:128 * (x + 1)]
                    nc.tensor.matmul(p3m_ps[:, 64 * x:64 * (x + 1)], p3jT_x,
                                     wj_pix_hi, start=False, stop=False)
                nc.tensor.matmul(p3m_ps[:], d3T, kron3, start=False, stop=True)
                nc.vector.tensor_tensor(
                    ixblk[:, b, :].rearrange("p (x c) -> p x c", c=C),
                    p3m_ps[:].rearrange("p (x c) -> p x c", c=C),
                    i1blk[:, b, C:2 * C][:, None, :].to_broadcast((128, 3, C)),
                    ALU.mult)

                # segment-sum into transposed accumulator (zeroed at window
                # start; closed after the last tile). Layout: cols 0:128 =
                # i1.T; cols 128+128x : 256+128x = ix_x.T on partitions 0:64.
                nc.tensor.matmul(acc[:, 0:128], i1blk[:, b, :], O[:],
                                 start=False, stop=False)
                for x in range(3):
                    nc.tensor.matmul(
                        acc[0:64, 128 + 128 * x:256 + 128 * x],
                        ixblk[:, b, 64 * x:64 * (x + 1)], O[:],
                        start=False, stop=False)

                if b == BT - 1 or t == TT - 1:
                    nbt = b + 1
                    if "noout" not in ablate:
                        nc.gpsimd.dma_start(
                            i1_d[blk0 * 128:(blk0 + nbt) * 128, :]
                            .rearrange("(b p) f -> p b f", b=nbt),
                            i1blk[:, 0:nbt, :])
                        nc.gpsimd.dma_start(
                            ix_d[blk0 * 128:(blk0 + nbt) * 128, :]
                            .rearrange("(b p) f -> p b f", b=nbt),
                            ixblk[:, 0:nbt, :])
                    i1blk = None
                    ixblk = None

            # close the acc accumulation group (adds zeros, full region)
            nc.tensor.matmul(acc[:], c16[0:1, 897:1025], c16[0:1, 897:1409],
                             start=False, stop=True)

            if "nophaseb" in ablate:
                continue
            pb_lvl = int(os.environ.get("GCB_PB_LVL", "9"))
            # ---- phase B for window m (fp32) ----
            accS = sb2.tile([128, 512], f32, name="accS")
            nc.scalar.activation(accS[:], acc[:], AF.Copy)
            if pb_lvl < 2:
                continue
            accT1 = accS[:, 0:128]
            a3 = [accS[0:64, 128 + 128 * x:256 + 128 * x] for x in range(3)]

            pn_ps = ps.tile([64, 128], f32, name="pn_ps", tag="pa")
            nc.tensor.matmul(pn_ps[:], W_pp, accT1, start=True, stop=True)
            p1newT = sb2.tile([64, 128], f32, name="p1newT")
            nc.scalar.activation(p1newT[:], pn_ps[:], AF.Tanh)
            if pb_lvl < 3:
                continue

            p3mB = ps.tile([128, 192], f32, name="p3mB", tag="pa")
            for x in range(3):
                nc.tensor.matmul(p3mB[:, 64 * x:64 * (x + 1)], a3[x], W_ppx,
                                 start=True, stop=True,
                                 skip_group_check=x > 0)
            if pb_lvl < 4:
                continue
            A_ps = ps.tile([64, 384], f32, name="A_ps", tag="pa")
            B_ps = ps.tile([64, 384], f32, name="B_ps", tag="pa")
            for x in range(3):
                nc.tensor.matmul(A_ps[:, 128 * x:128 * (x + 1)], Wi2, a3[x],
                                 start=True, stop=True, skip_group_check=x > 0)
                nc.tensor.matmul(B_ps[:, 128 * x:128 * (x + 1)], Wj2, a3[x],
                                 start=True, stop=True, skip_group_check=x > 0)
            A_sb = sb2.tile([64, 384], f32, name="A_sb")
            nc.scalar.activation(A_sb[:], A_ps[:], AF.Copy)
            ABm = sb2.tile([64, 384], f32, name="ABm")
            nc.vector.tensor_tensor(ABm[:], A_sb[:], B_ps[:], ALU.mult)
            dottedT = sb2.tile([64, 128], f32, name="dottedT")
            nc.vector.tensor_reduce(
                dottedT[:], ABm[:].rearrange("p (x a) -> p a x", x=3),
                AX.X, ALU.add)

            if pb_lvl < 5:
                continue
            t1_ps = ps.tile([128, 128], f32, name="t1_ps", tag="pa")
            nc.tensor.matmul(t1_ps[:], W_pp1_top, p1newT[:],
                             start=True, stop=False)
            nc.tensor.matmul(t1_ps[:], W_pp1_bot, dottedT[:],
                             start=False, stop=True)
            p1t1T = sb2.tile([128, 128], f32, name="p1t1T")
            nc.scalar.activation(p1t1T[:], t1_ps[:], AF.Tanh, bias=b_pp1)

            if pb_lvl < 6:
                continue
            pouts = sb2.tile([128, 320], f32, name="pouts")
            id64 = id32[0:64, 0:64]
            # one full transpose gives p1_out (cols 0:64) and s3 (cols 64:128)
            trF_ps = ps.tile([128, 128], f32, name="trF_ps", tag="pa")
            nc.tensor.transpose(trF_ps[:], p1t1T[:], id32)
            nc.scalar.activation(pouts[:, 0:64], trF_ps[:, 0:64], AF.Copy)
            s3t = sb2.tile([128, 64], f32, name="s3t")
            nc.scalar.activation(s3t[:], trF_ps[:, 64:128], AF.Copy)
            tr2_ps = ps.tile([128, 64], f32, name="tr2_ps", tag="pa")
            nc.tensor.transpose(tr2_ps[:], dottedT[:], id64)
            nc.scalar.activation(pouts[:, 64:128], tr2_ps[:], AF.Copy)
            nc.vector.tensor_tensor(
                pouts[:, 128:320].rearrange("p (x c) -> p x c", c=C),
                p3mB[:].rearrange("p (x c) -> p x c", c=C),
                s3t[:, None, :].to_broadcast((128, 3, C)), ALU.mult)
            nc.sync.dma_start(pb_d[m], pouts[:])

    nc.finalize()
    return nc


# ---------------------------------------------------------------- top level

def kernel(ind_2, p1, p3, d3, basis, W_pi, b_pi, W_ii, W_pp,
           wi_pix, wj_pix, W_ppx, wi_dot, wj_dot, W_pp1, b_pp1):
    ind_2 = np.asarray(ind_2)
    p1 = np.asarray(p1, np.float32)
    p3 = np.asarray(p3, np.float32)
    d3 = np.asarray(d3, np.float32)
    basis = np.asarray(basis, np.float32)
    W_pi = np.asarray(W_pi, np.float32)
    b_pi = np.asarray(b_pi, np.float32)
    W_ii = np.asarray(W_ii, np.float32)
    W_pp = np.asarray(W_pp, np.float32)
    wi_pix = np.asarray(wi_pix, np.float32)
    wj_pix = np.asarray(wj_pix, np.float32)
    W_ppx = np.asarray(W_ppx, np.float32)
    wi_dot = np.asarray(wi_dot, np.float32)
    wj_dot = np.asarray(wj_dot, np.float32)
    W_pp1 = np.asarray(W_pp1, np.float32)
    b_pp1 = np.asarray(b_pp1, np.float32)

    n_atoms = p1.shape[0]
    plan = plan_shards(ind_2, n_atoms, NCORES)
    na_pad = plan["NA_PAD"]
    p1_pad = np.zeros((na_pad, C), np.float32)
    p1_pad[:n_atoms] = p1
    p3_pad = np.zeros((na_pad, 3, C), np.float32)
    p3_pad[:n_atoms] = p3

    c16, c32 = prep_consts(W_pi, b_pi, W_ii, W_pp, wi_pix, wj_pix, W_ppx,
                           wi_dot, wj_dot, W_pp1, b_pp1)
    in_maps = []
    for k in range(NCORES):
        blob, wtab = prep_core_inputs(plan, k, ind_2, p1_pad, p3_pad, d3,
                                      basis, W_pi, b_pi, wi_pix)
        in_maps.append(dict(blob=blob, wtab=wtab, c16=c16, c32=c32))

    nc = build_kernel(plan["NWIN"], plan["TT"], plan["tile_slot"], plan["T_m"])
    global LAST_NC, LAST_IN_MAPS
    LAST_NC = nc
    LAST_IN_MAPS = in_maps
    import os
    if os.environ.get("GCB_SIM"):
        from concourse.bass_interp import CoreSim

        class _R:
            results = []
        res = _R()
        for k in range(int(os.environ.get("GCB_SIM_CORES", "1"))):
            sim = CoreSim(nc)
            for nm, arr in in_maps[k].items():
                sim.tensor(nm)[:] = arr
            sim.simulate()
            res.results.append({nm: np.array(sim.tensor(nm))
                                for nm in ("i1_out", "ix_out", "pb_out")})
        while len(res.results) < NCORES:
            res.results.append(res.results[-1])
    else:
        kw = {}
        if os.environ.get("GCB_TRACE"):
            kw["trace"] = True
        res = run_bass_kernel_spmd(nc, in_maps, core_ids=list(range(NCORES)), **kw)
        global LAST_EXEC_NS, LAST_RES
        LAST_EXEC_NS = getattr(res, "exec_time_ns", None)
        LAST_RES = res

    # ---- host reassembly ----
    n_pairs = ind_2.shape[0]
    i1_full = np.zeros((n_pairs, 2 * C), np.float32)
    ix_full = np.zeros((n_pairs, 3, C), np.float32)
    p1o = np.zeros((na_pad, C), np.float32)
    p3o = np.zeros((na_pad, 3, C), np.float32)
    doto = np.zeros((na_pad, C), np.float32)
    for k in range(NCORES):
        r = res.results[k]
        sel = plan["pair_sel"][k]
        v = sel >= 0
        i1_full[sel[v]] = r["i1_out"][v]
        ix_full[sel[v]] = r["ix_out"].reshape(-1, 3, C)[v]
        for m in range(plan["NWIN"]):
            w = plan["core_wins"][k][m]
            if w < 0:
                continue
            a0 = w * WIN
            pb = r["pb_out"][m]
            p1o[a0:a0 + WIN] = pb[:, 0:64]
            doto[a0:a0 + WIN] = pb[:, 64:128]
            p3o[a0:a0 + WIN] = pb[:, 128:320].reshape(WIN, 3, C)
    return (p1o[:n_atoms], p3o[:n_atoms], doto[:n_atoms], i1_full, ix_full)
